# revision 1
# baseline (speedup 1.0000x reference)
import sys

for p in ("/opt/trn_rl_repo",):
    if p not in sys.path:
        sys.path.insert(0, p)

import numpy as np

# Problem constants (hardcoded per contract)
B, F, E, U, H = 4096, 39, 64, 256, 8
DH = U // H
NCORES = 8
BC = B // NCORES          # 512 samples per core
T = BC * F                # 19968 tokens per core
TILE_N = 512              # tokens per matmul tile
NT = T // TILE_N          # 39 tiles
KA = E + 1                # augmented contraction dim (bias row)

_CACHE = {}


def _build_program():
    import concourse.bacc as bacc
    import concourse.mybir as mybir
    from concourse.tile import TileContext

    fp32 = mybir.dt.float32
    fp16 = mybir.dt.float16
    Relu = mybir.ActivationFunctionType.Relu

    nc = bacc.Bacc(None, target_bir_lowering=False)
    embT = nc.dram_tensor("embT", (KA, T), fp32, kind="ExternalInput")
    Waug = nc.dram_tensor("Waug", (KA, 3 * U), fp32, kind="ExternalInput")
    qkv = nc.dram_tensor("qkv", (3 * U, T), fp16, kind="ExternalOutput")

    with TileContext(nc) as tc:
        with (
            tc.tile_pool(name="wp", bufs=1) as wp,
            tc.tile_pool(name="inp", bufs=3) as inp,
            tc.tile_pool(name="ps", bufs=4, space="PSUM") as ps,
            tc.tile_pool(name="outp", bufs=4) as outp,
        ):
            w_sb = wp.tile([KA, 3 * U], fp32)
            nc.sync.dma_start(out=w_sb[:], in_=Waug[:])
            for t in range(NT):
                x_sb = inp.tile([KA, TILE_N], fp32)
                nc.sync.dma_start(
                    out=x_sb[:], in_=embT[:, t * TILE_N:(t + 1) * TILE_N]
                )
                for m in range(6):
                    acc = ps.tile([128, TILE_N], fp32)
                    nc.tensor.matmul(
                        acc[:],
                        w_sb[:, m * 128:(m + 1) * 128],
                        x_sb[:],
                        start=True,
                        stop=True,
                    )
                    y_sb = outp.tile([128, TILE_N], fp16)
                    nc.scalar.activation(y_sb[:], acc[:], Relu)
                    nc.sync.dma_start(
                        out=qkv[m * 128:(m + 1) * 128, t * TILE_N:(t + 1) * TILE_N],
                        in_=y_sb[:],
                    )
    nc.compile()
    return nc


def _get_program():
    if "nc" not in _CACHE:
        _CACHE["nc"] = _build_program()
    return _CACHE["nc"]


def kernel(feature_ids, emb_table, Wq, bq, Wk, bk, Wv, bv, Wp, bp):
    from concourse.bass_utils import run_bass_kernel_spmd

    feature_ids = np.asarray(feature_ids)
    emb_table = np.asarray(emb_table, dtype=np.float32)

    # Augmented weight: [E+1, 3U], last row = biases (bias folded into matmul)
    W_all = np.concatenate([Wq, Wk, Wv], axis=1).astype(np.float32)      # [64, 768]
    b_all = np.concatenate([bq, bk, bv], axis=0).astype(np.float32)      # [768]
    Waug = np.concatenate([W_all, b_all[None, :]], axis=0)               # [65, 768]
    Waug = np.ascontiguousarray(Waug)

    # Per-core host-side shard + embedding gather (data-parallel over batch)
    in_maps = []
    for c in range(NCORES):
        ids_c = feature_ids[c * BC:(c + 1) * BC]                          # [512, 39]
        emb_c = emb_table[ids_c.astype(np.int64)]                         # [512, 39, 64]
        embT_c = emb_c.reshape(T, E).T                                    # [64, 19968]
        embT_aug = np.concatenate(
            [embT_c, np.ones((1, T), np.float32)], axis=0
        )                                                                 # [65, 19968]
        in_maps.append(
            {"embT": np.ascontiguousarray(embT_aug), "Waug": Waug}
        )

    nc = _get_program()
    res = run_bass_kernel_spmd(nc, in_maps, list(range(NCORES)))

    # Host epilogue: attention + softmax + output head (numpy, fp32),
    # parallelized across cores (numpy releases the GIL for these ops).
    logits = np.empty((B, 1), np.float32)
    Wp = np.asarray(Wp, dtype=np.float32)
    bp = np.asarray(bp, dtype=np.float32)
    scale = 1.0 / np.sqrt(np.float32(DH))

    def epilogue(c):
        qkv_c = np.asarray(res.results[c]["qkv"]).astype(np.float32)      # [768, 19968]

        def proj(i):
            x = qkv_c[i * U:(i + 1) * U]                                  # [256, 19968]
            x = x.reshape(U, BC, F).transpose(1, 2, 0)                    # [BC, F, U]
            return x.reshape(BC, F, H, DH).transpose(0, 2, 1, 3)          # [BC, H, F, DH]

        q, k, v = proj(0), proj(1), proj(2)
        scores = (q @ k.transpose(0, 1, 3, 2)) * scale
        scores -= scores.max(axis=-1, keepdims=True)
        e = np.exp(scores)
        attn = e / e.sum(axis=-1, keepdims=True)
        out = attn @ v
        out = np.maximum(out.transpose(0, 2, 1, 3).reshape(BC, F * U), 0.0)
        logits[c * BC:(c + 1) * BC] = out @ Wp + bp

    from concurrent.futures import ThreadPoolExecutor

    with ThreadPoolExecutor(max_workers=NCORES) as ex:
        list(ex.map(epilogue, range(NCORES)))
    return logits



# revision 13
# speedup vs baseline: 14.7387x; 14.7387x over previous
import sys

for p in ("/opt/trn_rl_repo",):
    if p not in sys.path:
        sys.path.insert(0, p)

import numpy as np

# Problem constants (hardcoded per contract)
B, F, E, U, H = 4096, 39, 64, 256, 8
DH = U // H               # 32 head dim
NCORES = 8
BC = B // NCORES          # 512 samples per core
T = BC * F                # 19968 tokens per core
SPB = 8                   # samples per block
NBLK = BC // SPB          # 64 blocks
TB = SPB * F              # 312 tokens per block
NPAIR = SPB // 2          # 4 sample-pairs per block
KA = E + 1                # contraction dim with bias row
VA = DH + 1               # v cols per head (+1 denominator ones-column)
UV = H * VA               # 264
SCALE = 1.0 / float(np.sqrt(np.float32(DH)))

_CACHE = {}


def _build_program(n_blocks=NBLK, max_unroll=4):
    import concourse.bacc as bacc
    import concourse.bass as bass
    import concourse.mybir as mybir
    from concourse.tile import TileContext

    fp32 = mybir.dt.float32
    fp16 = mybir.dt.float16
    Relu = mybir.ActivationFunctionType.Relu
    Exp = mybir.ActivationFunctionType.Exp
    Mult = mybir.AluOpType.mult
    Add = mybir.AluOpType.add
    Max = mybir.AluOpType.max
    AxX = mybir.AxisListType.X

    TT = n_blocks * TB

    nc = bacc.Bacc(None, target_bir_lowering=False)
    emb = nc.dram_tensor("embT", (E, TT), fp16, kind="ExternalInput")
    wqk = nc.dram_tensor("Wqk", (KA, 2 * U), fp16, kind="ExternalInput")
    wv = nc.dram_tensor("Wv", (KA, UV), fp16, kind="ExternalInput")
    wp = nc.dram_tensor("Wp", (128, UV), fp16, kind="ExternalInput")
    bp2 = nc.dram_tensor("bp2", (2, 1), fp32, kind="ExternalInput")
    out = nc.dram_tensor("out", (2, n_blocks * NPAIR), fp32, kind="ExternalOutput")

    with TileContext(nc) as tc:
        with (
            tc.tile_pool(name="const", bufs=1) as cp,
            tc.tile_pool(name="xin", bufs=3) as xp,
            tc.tile_pool(name="qk", bufs=2) as qkpool,
            tc.tile_pool(name="attn", bufs=2) as ap,
            tc.tile_pool(name="qkps", bufs=1, space="PSUM") as qkps,
            tc.tile_pool(name="vps", bufs=1, space="PSUM") as vpsp,
            tc.tile_pool(name="sps", bufs=1, space="PSUM") as spsp,
            tc.tile_pool(name="ops", bufs=1, space="PSUM") as opsp,
        ):
            # --- constants / weights (persistent) ---
            wqk_sb = cp.tile([KA, 2 * U], fp16)
            nc.sync.dma_start(out=wqk_sb[:], in_=wqk[:])
            wv_sb = cp.tile([KA, UV], fp16)
            nc.sync.dma_start(out=wv_sb[:], in_=wv[:])
            wp_sb = cp.tile([128, H, VA], fp16)
            nc.sync.dma_start(out=wp_sb[:], in_=wp[:].rearrange("p (h v) -> p h v", v=VA))
            bp_sb = cp.tile([2, 1], fp32)
            nc.sync.dma_start(out=bp_sb[:], in_=bp2[:])
            onesAB = cp.tile([128, 2], fp32)
            nc.gpsimd.memset(onesAB[:], 0.0)
            nc.gpsimd.memset(onesAB[0:F, 0:1], 1.0)
            nc.gpsimd.memset(onesAB[64:64 + F, 1:2], 1.0)
            logblk = cp.tile([128, NPAIR], fp32)
            nc.gpsimd.memset(logblk[:], 0.0)
            logits_sb = cp.tile([2, n_blocks * NPAIR], fp32)

            def block_body(i):
                # i: block index (python int or loop register)
                x = xp.tile([KA, TB], fp16, tag="x")
                nc.sync.dma_start(out=x[0:E, :], in_=emb[:, bass.ds(i * TB, TB)])
                nc.gpsimd.memset(x[E:KA, :], 1.0)

                # q/k projections: psum [128, TB] per 128-wide u-slice
                qk_sb = []
                for m in range(4):
                    ps = qkps.tile([128, TB], fp32, tag="qkps")
                    nc.tensor.matmul(
                        ps[:], wqk_sb[:, m * 128:(m + 1) * 128], x[:],
                        start=True, stop=True,
                    )
                    sb = qkpool.tile([128, TB], fp16, tag=f"qk{m}")
                    nc.scalar.activation(sb[:], ps[:], Relu)
                    qk_sb.append(sb)
                q_lo, q_hi, k_lo, k_hi = qk_sb

                for p_ in range(NPAIR):
                    pair = ((2 * p_, 0), (2 * p_ + 1, 64))
                    # v projection (token-major, +ones col per head), one psum
                    # tile per sample; K=65 -> all v MMs share row groups 0-2
                    # (serialized on PE), outputs at partition base 0.
                    vt_s = []
                    for si, (a, base) in enumerate(pair):
                        vps = vpsp.tile([F, H, VA], fp32, tag=f"vps{si}")
                        nc.tensor.matmul(
                            vps[:, :, :].rearrange("p h v -> p (h v)"),
                            x[:, a * F:(a + 1) * F], wv_sb[:],
                            start=True, stop=True,
                        )
                        vt = ap.tile([F, H, VA], fp16, tag=f"vt{si}")
                        nc.scalar.activation(vt[:], vps[:], Relu)
                        vt_s.append(vt)

                    # scores^T = k q^T: one psum bank per PE row group
                    # (heads h and h+4 share a row group -> serialized, safe)
                    sgrp = []
                    for rg in range(4):
                        sg = spsp.tile([F, 2, 2, F], fp32, tag=f"sg{rg}")
                        sgrp.append(sg)
                    for si, (a, base) in enumerate(pair):
                        for h in range(H):
                            kt, qt = (k_lo, q_lo) if h < 4 else (k_hi, q_hi)
                            rg, hh = h % 4, h // 4
                            hb = rg * DH
                            nc.tensor.matmul(
                                sgrp[rg][:, hh, si, :],
                                kt[hb:hb + DH, a * F:(a + 1) * F],
                                qt[hb:hb + DH, a * F:(a + 1) * F],
                                start=True, stop=True,
                                tile_position=(hb, 0),
                            )
                    exg = []
                    for rg in range(4):
                        ex = ap.tile([F, 2, 2, F], fp16, tag=f"ex{rg}")
                        nc.scalar.activation(ex[:], sgrp[rg][:], Exp, scale=SCALE)
                        exg.append(ex)

                    # out_unnorm = exp^T @ v_aug (last col = denominator).
                    # K=39 -> all AV MMs share row groups 0-1 (serialized), so
                    # packing samples at partition bases 0/64 of one bank is safe.
                    ops_ = opsp.tile([128, H, VA], fp32, tag="opsum")
                    for si, (a, base) in enumerate(pair):
                        for h in range(H):
                            rg, hh = h % 4, h // 4
                            nc.tensor.matmul(
                                ops_[base:base + F, h, :],
                                exg[rg][:, hh, si, :],
                                vt_s[si][:, h, :],
                                start=True, stop=True,
                            )
                    rc = ap.tile([128, H], fp32, tag="rc")
                    t = ap.tile([128, H, DH], fp32, tag="t")
                    part = ap.tile([128, H], fp32, tag="part")
                    t2 = ap.tile([128, H], fp32, tag="t2")
                    for base in (0, 64):
                        nc.vector.reciprocal(
                            rc[base:base + F], ops_[base:base + F, :, DH]
                        )
                        # t = relu(out_unnorm) * Wp   (relu+mul fused)
                        nc.vector.scalar_tensor_tensor(
                            out=t[base:base + F],
                            in0=ops_[base:base + F, :, 0:DH],
                            scalar=0.0,
                            in1=wp_sb[base:base + F, :, 0:DH],
                            op0=Max,
                            op1=Mult,
                        )
                        # partial[f, h] = sum_d t
                        nc.vector.tensor_reduce(
                            out=part[base:base + F],
                            in_=t[base:base + F],
                            axis=AxX,
                            op=Add,
                        )
                        # logit partials per f-row: sum_h partial * (1/denom)
                        nc.vector.tensor_mul(
                            t2[base:base + F], part[base:base + F], rc[base:base + F]
                        )
                        nc.vector.tensor_reduce(
                            out=logblk[base:base + F, p_:p_ + 1],
                            in_=t2[base:base + F],
                            axis=AxX,
                            op=Add,
                        )

                # per-block logits: sum partials over the 39 f-rows
                fps = opsp.tile([2, NPAIR], fp32, tag="opsum")
                nc.tensor.matmul(fps[:], onesAB[:], logblk[:], start=True, stop=True)
                nc.scalar.add(
                    logits_sb[:, bass.ds(i * NPAIR, NPAIR)], fps[:], bp_sb[0:2, 0:1]
                )

            if max_unroll >= n_blocks:
                for i in range(n_blocks):
                    block_body(i)
            else:
                tc.For_i_unrolled(0, n_blocks, 1, block_body, max_unroll=max_unroll)

            nc.sync.dma_start(out=out[:], in_=logits_sb[:])

    nc.compile()
    return nc


def _get_program():
    if "nc" not in _CACHE:
        _CACHE["nc"] = _build_program()
    return _CACHE["nc"]


def _prep_weights(Wq, bq, Wk, bk, Wv, bv, Wp, bp):
    f32 = np.float32
    Wqk = np.concatenate(
        [
            np.concatenate([Wq.astype(f32), bq.astype(f32)[None, :]], axis=0),
            np.concatenate([Wk.astype(f32), bk.astype(f32)[None, :]], axis=0),
        ],
        axis=1,
    )  # [65, 512]
    Wva = np.zeros((KA, UV), f32)
    for h in range(H):
        Wva[:E, h * VA:h * VA + DH] = Wv[:, h * DH:(h + 1) * DH]
        Wva[E, h * VA:h * VA + DH] = bv[h * DH:(h + 1) * DH]
        Wva[E, h * VA + DH] = 1.0  # ones-column -> denominator
    WpM = np.zeros((128, UV), f32)
    wp3 = Wp.astype(f32).reshape(F, H, DH)  # idx f*256 + h*32 + d
    for base in (0, 64):
        for h in range(H):
            WpM[base:base + F, h * VA:h * VA + DH] = wp3[:, h, :]
    bp2 = np.full((2, 1), np.float32(bp[0]), f32)
    return (
        np.ascontiguousarray(Wqk.astype(np.float16)),
        np.ascontiguousarray(Wva.astype(np.float16)),
        np.ascontiguousarray(WpM.astype(np.float16)),
        bp2,
    )


def kernel(feature_ids, emb_table, Wq, bq, Wk, bk, Wv, bv, Wp, bp):
    from concourse.bass_utils import run_bass_kernel_spmd

    feature_ids = np.asarray(feature_ids)
    emb_table = np.asarray(emb_table, dtype=np.float32)
    Wqk, Wva, WpM, bp2 = _prep_weights(
        np.asarray(Wq), np.asarray(bq), np.asarray(Wk), np.asarray(bk),
        np.asarray(Wv), np.asarray(bv), np.asarray(Wp), np.asarray(bp),
    )

    in_maps = []
    for c in range(NCORES):
        ids_c = feature_ids[c * BC:(c + 1) * BC].astype(np.int64)   # [512, 39]
        emb_c = emb_table[ids_c]                                    # [512, 39, 64]
        embT = np.ascontiguousarray(
            emb_c.reshape(T, E).T.astype(np.float16)
        )                                                           # [64, 19968]
        in_maps.append(
            {"embT": embT, "Wqk": Wqk, "Wv": Wva, "Wp": WpM, "bp2": bp2}
        )
    _CACHE["last_in_maps"] = in_maps

    nc = _get_program()
    res = run_bass_kernel_spmd(nc, in_maps, list(range(NCORES)))

    logits = np.empty((B, 1), np.float32)
    for c in range(NCORES):
        o = np.asarray(res.results[c]["out"])                       # [2, 256]
        logits[c * BC:(c + 1) * BC, 0] = (
            o.reshape(2, NBLK, NPAIR).transpose(1, 2, 0).reshape(BC)
        )
    return logits


# revision 19
# speedup vs baseline: 19.3057x; 1.3099x over previous
import sys

for p in ("/opt/trn_rl_repo",):
    if p not in sys.path:
        sys.path.insert(0, p)

import numpy as np

# Problem constants (hardcoded per contract)
B, F, E, U, H = 4096, 39, 64, 256, 8
DH = U // H               # 32 head dim
NCORES = 8
BC = B // NCORES          # 512 samples per core
T = BC * F                # 19968 tokens per core
SPB = 8                   # samples per block
NBLK = BC // SPB          # 64 blocks
TB = SPB * F              # 312 tokens per block
NPAIR = SPB // 2          # 4 sample-pairs per block
KA = E + 1                # contraction dim with bias row
VA = DH + 1               # v cols per head (+1 denominator ones-column)
UV = H * VA               # 264
SCALE = 1.0 / float(np.sqrt(np.float32(DH)))

_CACHE = {}


def _build_program(n_blocks=NBLK, max_unroll=4):
    import concourse.bacc as bacc
    import concourse.bass as bass
    import concourse.mybir as mybir
    from concourse.tile import TileContext

    fp32 = mybir.dt.float32
    fp16 = mybir.dt.float16
    i8 = mybir.dt.int8
    Relu = mybir.ActivationFunctionType.Relu
    Exp = mybir.ActivationFunctionType.Exp
    Mult = mybir.AluOpType.mult
    Add = mybir.AluOpType.add
    Max = mybir.AluOpType.max
    AxX = mybir.AxisListType.X

    TT = n_blocks * TB

    nc = bacc.Bacc(None, target_bir_lowering=False)
    emb = nc.dram_tensor("embT", (E, TT), i8, kind="ExternalInput")
    wqk = nc.dram_tensor("Wqk", (KA, 2 * U), fp16, kind="ExternalInput")
    wv = nc.dram_tensor("Wv", (KA, UV), fp16, kind="ExternalInput")
    wp = nc.dram_tensor("Wp", (F, UV), fp16, kind="ExternalInput")
    bp2 = nc.dram_tensor("bp2", (2, 1), fp32, kind="ExternalInput")
    out = nc.dram_tensor("out", (2, n_blocks * NPAIR), fp32, kind="ExternalOutput")

    with TileContext(nc) as tc:
        with (
            tc.tile_pool(name="const", bufs=1) as cp,
            tc.tile_pool(name="xin", bufs=3) as xp,
            tc.tile_pool(name="qk", bufs=2) as qkpool,
            tc.tile_pool(name="attn", bufs=2) as ap,
            tc.tile_pool(name="qkps", bufs=1, space="PSUM") as qkps,
            tc.tile_pool(name="vps", bufs=1, space="PSUM") as vpsp,
            tc.tile_pool(name="sps", bufs=1, space="PSUM") as spsp,
            tc.tile_pool(name="ops", bufs=1, space="PSUM") as opsp,
        ):
            # --- constants / weights (persistent) ---
            wqk_sb = cp.tile([KA, 2 * U], fp16)
            nc.sync.dma_start(out=wqk_sb[:], in_=wqk[:])
            wv_sb = cp.tile([KA, UV], fp16)
            nc.sync.dma_start(out=wv_sb[:], in_=wv[:])
            wp_sb = cp.tile([128, H, VA], fp16)
            for base in (0, 64):
                nc.sync.dma_start(
                    out=wp_sb[base:base + F],
                    in_=wp[:].rearrange("p (h v) -> p h v", v=VA),
                )
            bp_sb = cp.tile([2, 1], fp32)
            nc.sync.dma_start(out=bp_sb[:], in_=bp2[:])
            onesAB = cp.tile([128, 2], fp32)
            nc.gpsimd.memset(onesAB[:], 0.0)
            nc.gpsimd.memset(onesAB[0:F, 0:1], 1.0)
            nc.gpsimd.memset(onesAB[64:64 + F, 1:2], 1.0)
            logblk = cp.tile([128, NPAIR], fp32)
            nc.gpsimd.memset(logblk[:], 0.0)
            logits_sb = cp.tile([2, n_blocks * NPAIR], fp32)

            def block_body(i):
                # i: block index (python int or loop register)
                xi = xp.tile([E, TB], i8, tag="xi")
                nc.sync.dma_start(out=xi[:], in_=emb[:, bass.ds(i * TB, TB)])
                x = xp.tile([KA, TB], fp16, tag="x")
                # int8 -> fp16 (quant scale is folded into Wqk/Wv on host)
                nc.vector.tensor_copy(out=x[0:E, :], in_=xi[:])
                nc.gpsimd.memset(x[E:KA, :], 1.0)

                # q/k projections: psum [128, TB] per 128-wide u-slice
                qk_sb = []
                for m in range(4):
                    ps = qkps.tile([128, TB], fp32, tag="qkps")
                    nc.tensor.matmul(
                        ps[:], wqk_sb[:, m * 128:(m + 1) * 128], x[:],
                        start=True, stop=True,
                    )
                    sb = qkpool.tile([128, TB], fp16, tag=f"qk{m}")
                    nc.scalar.activation(sb[:], ps[:], Relu)
                    qk_sb.append(sb)
                q_lo, q_hi, k_lo, k_hi = qk_sb

                for p_ in range(NPAIR):
                    pair = ((2 * p_, 0), (2 * p_ + 1, 64))
                    # v projection (token-major, +ones col per head), one psum
                    # tile per sample; K=65 -> all v MMs share row groups 0-2
                    # (serialized on PE), outputs at partition base 0.
                    vt_s = []
                    for si, (a, base) in enumerate(pair):
                        vps = vpsp.tile([F, H, VA], fp32, tag=f"vps{si}")
                        nc.tensor.matmul(
                            vps[:, :, :].rearrange("p h v -> p (h v)"),
                            x[:, a * F:(a + 1) * F], wv_sb[:],
                            start=True, stop=True,
                        )
                        vt = ap.tile([F, H, VA], fp16, tag=f"vt{si}")
                        nc.scalar.activation(vt[:], vps[:], Relu)
                        vt_s.append(vt)

                    # scores^T = k q^T: one psum bank per PE row group
                    # (heads h and h+4 share a row group -> serialized, safe)
                    sgrp = []
                    for rg in range(4):
                        sg = spsp.tile([F, 2, 2, F], fp32, tag=f"sg{rg}")
                        sgrp.append(sg)
                    for si, (a, base) in enumerate(pair):
                        for h in range(H):
                            kt, qt = (k_lo, q_lo) if h < 4 else (k_hi, q_hi)
                            rg, hh = h % 4, h // 4
                            hb = rg * DH
                            nc.tensor.matmul(
                                sgrp[rg][:, hh, si, :],
                                kt[hb:hb + DH, a * F:(a + 1) * F],
                                qt[hb:hb + DH, a * F:(a + 1) * F],
                                start=True, stop=True,
                                tile_position=(hb, 0),
                            )
                    exg = []
                    for rg in range(4):
                        ex = ap.tile([F, 2, 2, F], fp16, tag=f"ex{rg}")
                        nc.scalar.activation(ex[:], sgrp[rg][:], Exp, scale=SCALE)
                        exg.append(ex)

                    # out_unnorm = exp^T @ v_aug (last col = denominator).
                    # K=39 -> all AV MMs share row groups 0-1 (serialized), so
                    # packing samples at partition bases 0/64 of one bank is safe.
                    ops_ = opsp.tile([128, H, VA], fp32, tag="opsum")
                    for si, (a, base) in enumerate(pair):
                        for h in range(H):
                            rg, hh = h % 4, h // 4
                            nc.tensor.matmul(
                                ops_[base:base + F, h, :],
                                exg[rg][:, hh, si, :],
                                vt_s[si][:, h, :],
                                start=True, stop=True,
                            )
                    rc = ap.tile([128, H], fp32, tag="rc")
                    t = ap.tile([128, H, DH], fp32, tag="t")
                    part = ap.tile([128, H], fp32, tag="part")
                    t2 = ap.tile([128, H], fp32, tag="t2")
                    for base in (0, 64):
                        nc.vector.reciprocal(
                            rc[base:base + F], ops_[base:base + F, :, DH]
                        )
                        # t = relu(out_unnorm) * Wp   (relu+mul fused)
                        nc.vector.scalar_tensor_tensor(
                            out=t[base:base + F],
                            in0=ops_[base:base + F, :, 0:DH],
                            scalar=0.0,
                            in1=wp_sb[base:base + F, :, 0:DH],
                            op0=Max,
                            op1=Mult,
                        )
                        # partial[f, h] = sum_d t
                        nc.vector.tensor_reduce(
                            out=part[base:base + F],
                            in_=t[base:base + F],
                            axis=AxX,
                            op=Add,
                        )
                        # logit partials per f-row: sum_h partial * (1/denom)
                        nc.vector.tensor_mul(
                            t2[base:base + F], part[base:base + F], rc[base:base + F]
                        )
                        nc.vector.tensor_reduce(
                            out=logblk[base:base + F, p_:p_ + 1],
                            in_=t2[base:base + F],
                            axis=AxX,
                            op=Add,
                        )

                # per-block logits: sum partials over the 39 f-rows
                fps = opsp.tile([2, NPAIR], fp32, tag="opsum")
                nc.tensor.matmul(fps[:], onesAB[:], logblk[:], start=True, stop=True)
                nc.scalar.add(
                    logits_sb[:, bass.ds(i * NPAIR, NPAIR)], fps[:], bp_sb[0:2, 0:1]
                )

            if max_unroll >= n_blocks:
                for i in range(n_blocks):
                    block_body(i)
            else:
                tc.For_i_unrolled(0, n_blocks, 1, block_body, max_unroll=max_unroll)

            nc.sync.dma_start(out=out[:], in_=logits_sb[:])

    nc.compile()
    return nc


def _get_program():
    if "nc" not in _CACHE:
        _CACHE["nc"] = _build_program()
    return _CACHE["nc"]


def _prep_weights(Wq, bq, Wk, bk, Wv, bv, Wp, bp, emb_scale):
    # emb_scale: int8 dequant scale, folded into the E-rows of Wqk/Wva
    f32 = np.float32
    Wqk = np.concatenate(
        [
            np.concatenate([Wq.astype(f32) * emb_scale, bq.astype(f32)[None, :]], axis=0),
            np.concatenate([Wk.astype(f32) * emb_scale, bk.astype(f32)[None, :]], axis=0),
        ],
        axis=1,
    )  # [65, 512]
    Wva = np.zeros((KA, UV), f32)
    for h in range(H):
        Wva[:E, h * VA:h * VA + DH] = Wv[:, h * DH:(h + 1) * DH] * emb_scale
        Wva[E, h * VA:h * VA + DH] = bv[h * DH:(h + 1) * DH]
        Wva[E, h * VA + DH] = 1.0  # ones-column -> denominator
    WpM = np.zeros((F, UV), f32)
    wp3 = Wp.astype(f32).reshape(F, H, DH)  # idx f*256 + h*32 + d
    for h in range(H):
        WpM[:, h * VA:h * VA + DH] = wp3[:, h, :]
    bp2 = np.full((2, 1), np.float32(bp[0]), f32)
    return (
        np.ascontiguousarray(Wqk.astype(np.float16)),
        np.ascontiguousarray(Wva.astype(np.float16)),
        np.ascontiguousarray(WpM.astype(np.float16)),
        bp2,
    )


def kernel(feature_ids, emb_table, Wq, bq, Wk, bk, Wv, bv, Wp, bp):
    from concourse.bass_utils import run_bass_kernel_spmd

    feature_ids = np.asarray(feature_ids)
    emb_table = np.asarray(emb_table, dtype=np.float32)
    emb_scale = float(np.abs(emb_table).max()) / 127.0
    if emb_scale == 0.0:
        emb_scale = 1.0
    table_i8 = np.clip(
        np.round(emb_table / emb_scale), -127, 127
    ).astype(np.int8)
    Wqk, Wva, WpM, bp2 = _prep_weights(
        np.asarray(Wq), np.asarray(bq), np.asarray(Wk), np.asarray(bk),
        np.asarray(Wv), np.asarray(bv), np.asarray(Wp), np.asarray(bp),
        emb_scale,
    )

    in_maps = []
    for c in range(NCORES):
        ids_c = feature_ids[c * BC:(c + 1) * BC].astype(np.int64)   # [512, 39]
        emb_c = table_i8[ids_c]                                     # [512, 39, 64] int8
        embT = np.ascontiguousarray(emb_c.reshape(T, E).T)          # [64, 19968]
        in_maps.append(
            {"embT": embT, "Wqk": Wqk, "Wv": Wva, "Wp": WpM, "bp2": bp2}
        )
    _CACHE["last_in_maps"] = in_maps

    nc = _get_program()
    res = run_bass_kernel_spmd(nc, in_maps, list(range(NCORES)))

    logits = np.empty((B, 1), np.float32)
    for c in range(NCORES):
        o = np.asarray(res.results[c]["out"])                       # [2, 256]
        logits[c * BC:(c + 1) * BC, 0] = (
            o.reshape(2, NBLK, NPAIR).transpose(1, 2, 0).reshape(BC)
        )
    return logits


# revision 20
# speedup vs baseline: 22.2131x; 1.1506x over previous
import sys

for p in ("/opt/trn_rl_repo",):
    if p not in sys.path:
        sys.path.insert(0, p)

import numpy as np

# Problem constants (hardcoded per contract)
B, F, E, U, H = 4096, 39, 64, 256, 8
DH = U // H               # 32 head dim
NCORES = 8
BC = B // NCORES          # 512 samples per core
T = BC * F                # 19968 tokens per core
SPB = 8                   # samples per block
NBLK = BC // SPB          # 64 blocks
TB = SPB * F              # 312 tokens per block
NPAIR = SPB // 2          # 4 sample-pairs per block
KA = E + 1                # contraction dim with bias row
VA = DH + 1               # v cols per head (+1 denominator ones-column)
UV = H * VA               # 264
SCALE = 1.0 / float(np.sqrt(np.float32(DH)))

_CACHE = {}


def _build_program(n_blocks=NBLK, max_unroll=1):
    import concourse.bacc as bacc
    import concourse.bass as bass
    import concourse.mybir as mybir
    from concourse.tile import TileContext

    fp32 = mybir.dt.float32
    fp16 = mybir.dt.float16
    i8 = mybir.dt.int8
    Relu = mybir.ActivationFunctionType.Relu
    Exp = mybir.ActivationFunctionType.Exp
    Mult = mybir.AluOpType.mult
    Add = mybir.AluOpType.add
    Max = mybir.AluOpType.max
    AxX = mybir.AxisListType.X

    TT = n_blocks * TB

    nc = bacc.Bacc(None, target_bir_lowering=False)
    emb = nc.dram_tensor("embT", (E, TT), i8, kind="ExternalInput")
    wqk = nc.dram_tensor("Wqk", (KA, 2 * U), fp16, kind="ExternalInput")
    wv = nc.dram_tensor("Wv", (KA, UV), fp16, kind="ExternalInput")
    wp = nc.dram_tensor("Wp", (F, UV), fp16, kind="ExternalInput")
    bp2 = nc.dram_tensor("bp2", (2, 1), fp32, kind="ExternalInput")
    out = nc.dram_tensor("out", (2, n_blocks * NPAIR), fp32, kind="ExternalOutput")

    with TileContext(nc) as tc:
        with (
            tc.tile_pool(name="const", bufs=1) as cp,
            tc.tile_pool(name="xin", bufs=3) as xp,
            tc.tile_pool(name="qk", bufs=2) as qkpool,
            tc.tile_pool(name="attn", bufs=2) as ap,
            tc.tile_pool(name="qkps", bufs=1, space="PSUM") as qkps,
            tc.tile_pool(name="vps", bufs=1, space="PSUM") as vpsp,
            tc.tile_pool(name="sps", bufs=1, space="PSUM") as spsp,
            tc.tile_pool(name="ops", bufs=1, space="PSUM") as opsp,
        ):
            # --- constants / weights (persistent) ---
            wqk_sb = cp.tile([KA, 2 * U], fp16)
            nc.sync.dma_start(out=wqk_sb[:], in_=wqk[:])
            wv_sb = cp.tile([KA, UV], fp16)
            nc.sync.dma_start(out=wv_sb[:], in_=wv[:])
            wp_sb = cp.tile([128, H, VA], fp16)
            for base in (0, 64):
                nc.sync.dma_start(
                    out=wp_sb[base:base + F],
                    in_=wp[:].rearrange("p (h v) -> p h v", v=VA),
                )
            bp_sb = cp.tile([2, 1], fp32)
            nc.sync.dma_start(out=bp_sb[:], in_=bp2[:])
            onesAB = cp.tile([128, 2], fp32)
            nc.gpsimd.memset(onesAB[:], 0.0)
            nc.gpsimd.memset(onesAB[0:F, 0:1], 1.0)
            nc.gpsimd.memset(onesAB[64:64 + F, 1:2], 1.0)
            logblk = cp.tile([128, NPAIR], fp32)
            nc.gpsimd.memset(logblk[:], 0.0)
            logits_sb = cp.tile([2, n_blocks * NPAIR], fp32)

            def block_body(i):
                # i: block index (python int or loop register)
                xi = xp.tile([E, TB], i8, tag="xi")
                nc.sync.dma_start(out=xi[:], in_=emb[:, bass.ds(i * TB, TB)])
                x = xp.tile([KA, TB], fp16, tag="x")
                # int8 -> fp16 (quant scale is folded into Wqk/Wv on host)
                nc.vector.tensor_copy(out=x[0:E, :], in_=xi[:])
                nc.gpsimd.memset(x[E:KA, :], 1.0)

                # q/k projections: psum [128, TB] per 128-wide u-slice
                qk_sb = []
                for m in range(4):
                    ps = qkps.tile([128, TB], fp32, tag="qkps")
                    nc.tensor.matmul(
                        ps[:], wqk_sb[:, m * 128:(m + 1) * 128], x[:],
                        start=True, stop=True,
                    )
                    sb = qkpool.tile([128, TB], fp16, tag=f"qk{m}")
                    nc.scalar.activation(sb[:], ps[:], Relu)
                    qk_sb.append(sb)
                q_lo, q_hi, k_lo, k_hi = qk_sb

                for p_ in range(NPAIR):
                    pair = ((2 * p_, 0), (2 * p_ + 1, 64))
                    # v projection (token-major, +ones col per head), one psum
                    # tile per sample; K=65 -> all v MMs share row groups 0-2
                    # (serialized on PE), outputs at partition base 0.
                    vt_s = []
                    for si, (a, base) in enumerate(pair):
                        vps = vpsp.tile([F, H, VA], fp32, tag=f"vps{si}")
                        nc.tensor.matmul(
                            vps[:, :, :].rearrange("p h v -> p (h v)"),
                            x[:, a * F:(a + 1) * F], wv_sb[:],
                            start=True, stop=True,
                        )
                        vt = ap.tile([F, H, VA], fp16, tag=f"vt{si}")
                        nc.scalar.activation(vt[:], vps[:], Relu)
                        vt_s.append(vt)

                    # scores^T = k q^T: one psum bank per PE row group
                    # (heads h and h+4 share a row group -> serialized, safe)
                    sgrp = []
                    for rg in range(4):
                        sg = spsp.tile([F, 2, 2, F], fp32, tag=f"sg{rg}")
                        sgrp.append(sg)
                    for si, (a, base) in enumerate(pair):
                        for h in range(H):
                            kt, qt = (k_lo, q_lo) if h < 4 else (k_hi, q_hi)
                            rg, hh = h % 4, h // 4
                            hb = rg * DH
                            nc.tensor.matmul(
                                sgrp[rg][:, hh, si, :],
                                kt[hb:hb + DH, a * F:(a + 1) * F],
                                qt[hb:hb + DH, a * F:(a + 1) * F],
                                start=True, stop=True,
                                tile_position=(hb, 0),
                            )
                    exg = []
                    for rg in range(4):
                        ex = ap.tile([F, 2, 2, F], fp16, tag=f"ex{rg}")
                        nc.scalar.activation(ex[:], sgrp[rg][:], Exp, scale=SCALE)
                        exg.append(ex)

                    # out_unnorm = exp^T @ v_aug (last col = denominator).
                    # K=39 -> all AV MMs share row groups 0-1 (serialized), so
                    # packing samples at partition bases 0/64 of one bank is safe.
                    ops_ = opsp.tile([128, H, VA], fp32, tag="opsum")
                    for si, (a, base) in enumerate(pair):
                        for h in range(H):
                            rg, hh = h % 4, h // 4
                            nc.tensor.matmul(
                                ops_[base:base + F, h, :],
                                exg[rg][:, hh, si, :],
                                vt_s[si][:, h, :],
                                start=True, stop=True,
                            )
                    rc = ap.tile([128, H], fp32, tag="rc")
                    t = ap.tile([128, H, DH], fp32, tag="t")
                    part = ap.tile([128, H], fp32, tag="part")
                    t2 = ap.tile([128, H], fp32, tag="t2")
                    for base in (0, 64):
                        nc.vector.reciprocal(
                            rc[base:base + F], ops_[base:base + F, :, DH]
                        )
                        # t = relu(out_unnorm) * Wp   (relu+mul fused)
                        nc.vector.scalar_tensor_tensor(
                            out=t[base:base + F],
                            in0=ops_[base:base + F, :, 0:DH],
                            scalar=0.0,
                            in1=wp_sb[base:base + F, :, 0:DH],
                            op0=Max,
                            op1=Mult,
                        )
                        # partial[f, h] = sum_d t
                        nc.vector.tensor_reduce(
                            out=part[base:base + F],
                            in_=t[base:base + F],
                            axis=AxX,
                            op=Add,
                        )
                        # logit partials per f-row: sum_h partial * (1/denom)
                        nc.vector.tensor_mul(
                            t2[base:base + F], part[base:base + F], rc[base:base + F]
                        )
                        nc.vector.tensor_reduce(
                            out=logblk[base:base + F, p_:p_ + 1],
                            in_=t2[base:base + F],
                            axis=AxX,
                            op=Add,
                        )

                # per-block logits: sum partials over the 39 f-rows
                fps = opsp.tile([2, NPAIR], fp32, tag="opsum")
                nc.tensor.matmul(fps[:], onesAB[:], logblk[:], start=True, stop=True)
                nc.scalar.add(
                    logits_sb[:, bass.ds(i * NPAIR, NPAIR)], fps[:], bp_sb[0:2, 0:1]
                )

            if max_unroll >= n_blocks:
                for i in range(n_blocks):
                    block_body(i)
            else:
                tc.For_i_unrolled(0, n_blocks, 1, block_body, max_unroll=max_unroll)

            nc.sync.dma_start(out=out[:], in_=logits_sb[:])

    nc.compile()
    return nc


def _get_program():
    if "nc" not in _CACHE:
        _CACHE["nc"] = _build_program()
    return _CACHE["nc"]


def _prep_weights(Wq, bq, Wk, bk, Wv, bv, Wp, bp, emb_scale):
    # emb_scale: int8 dequant scale, folded into the E-rows of Wqk/Wva
    f32 = np.float32
    Wqk = np.concatenate(
        [
            np.concatenate([Wq.astype(f32) * emb_scale, bq.astype(f32)[None, :]], axis=0),
            np.concatenate([Wk.astype(f32) * emb_scale, bk.astype(f32)[None, :]], axis=0),
        ],
        axis=1,
    )  # [65, 512]
    Wva = np.zeros((KA, UV), f32)
    for h in range(H):
        Wva[:E, h * VA:h * VA + DH] = Wv[:, h * DH:(h + 1) * DH] * emb_scale
        Wva[E, h * VA:h * VA + DH] = bv[h * DH:(h + 1) * DH]
        Wva[E, h * VA + DH] = 1.0  # ones-column -> denominator
    WpM = np.zeros((F, UV), f32)
    wp3 = Wp.astype(f32).reshape(F, H, DH)  # idx f*256 + h*32 + d
    for h in range(H):
        WpM[:, h * VA:h * VA + DH] = wp3[:, h, :]
    bp2 = np.full((2, 1), np.float32(bp[0]), f32)
    return (
        np.ascontiguousarray(Wqk.astype(np.float16)),
        np.ascontiguousarray(Wva.astype(np.float16)),
        np.ascontiguousarray(WpM.astype(np.float16)),
        bp2,
    )


def kernel(feature_ids, emb_table, Wq, bq, Wk, bk, Wv, bv, Wp, bp):
    from concourse.bass_utils import run_bass_kernel_spmd

    feature_ids = np.asarray(feature_ids)
    emb_table = np.asarray(emb_table, dtype=np.float32)
    emb_scale = float(np.abs(emb_table).max()) / 127.0
    if emb_scale == 0.0:
        emb_scale = 1.0
    table_i8 = np.clip(
        np.round(emb_table / emb_scale), -127, 127
    ).astype(np.int8)
    Wqk, Wva, WpM, bp2 = _prep_weights(
        np.asarray(Wq), np.asarray(bq), np.asarray(Wk), np.asarray(bk),
        np.asarray(Wv), np.asarray(bv), np.asarray(Wp), np.asarray(bp),
        emb_scale,
    )

    in_maps = []
    for c in range(NCORES):
        ids_c = feature_ids[c * BC:(c + 1) * BC].astype(np.int64)   # [512, 39]
        emb_c = table_i8[ids_c]                                     # [512, 39, 64] int8
        embT = np.ascontiguousarray(emb_c.reshape(T, E).T)          # [64, 19968]
        in_maps.append(
            {"embT": embT, "Wqk": Wqk, "Wv": Wva, "Wp": WpM, "bp2": bp2}
        )
    _CACHE["last_in_maps"] = in_maps

    nc = _get_program()
    res = run_bass_kernel_spmd(nc, in_maps, list(range(NCORES)))

    logits = np.empty((B, 1), np.float32)
    for c in range(NCORES):
        o = np.asarray(res.results[c]["out"])                       # [2, 256]
        logits[c * BC:(c + 1) * BC, 0] = (
            o.reshape(2, NBLK, NPAIR).transpose(1, 2, 0).reshape(BC)
        )
    return logits
